# revision 8
# baseline (speedup 1.0000x reference)
"""CARAFE (content-aware upsampling) Trainium2 Bass kernel.

Problem: features [2,64,64,128] f32, masks [2,128,128,25] f32 ->
out [2,128,128,128] f32; kernel_size=5, 2x nearest upsample, per-pixel
softmax over the 25-tap window, weighted sum of the 5x5 low-res patch.

Formulation: for each 8x16 output-pixel tile the 25 taps of all 128
pixels live inside an 8x12 low-res feature region (96 pixels). The
whole tile is then ONE matmul on the tensor engine:

    out[pix, c] = sum_p expW[p, pix] * Freg[p, c] / denom[pix]

where expW is the exp of the raw mask logits scattered (host-side, pure
data movement) into the [96 region, 128 pix] layout with -1e4 fill
(exp -> 0). exp runs on the scalar engine over the scattered layout.
Denominators come from a separately-loaded COMPACT copy of the logits
([pixel, 25]) which is exp'd (one scalar instr), free-dim-reduced and
reciprocal'd on the vector engine -- identical f16 exp values, so the
softmax is exact.  The PSUM->SBUF normalize is one broadcast
tensor_tensor multiply per 4-tile chunk (f32 PSUM read, f16 write),
split between the vector and gpsimd engines to balance load.

All DRAM traffic is host-prearranged fully contiguous and f16 (output
converted to f32 host-side): per core 1 compact-mask load (205KB),
4 weight loads (197KB), 4 feature-region loads (197KB), 4 output
stores (256KB).

Sharding: 8 cores = batch (2) x 4 row-bands of 32 output rows.
"""

import os
import numpy as np
from contextlib import ExitStack

import concourse.bacc as bacc
import concourse.bass as bass
import concourse.tile as tile
import concourse.mybir as mybir
from concourse import bass_utils

B, H, W, MC = 2, 128, 128, 25
LH, LW, C = 64, 64, 128
K5 = 5
TILE_U, TILE_V = 8, 16     # output tile: 8 rows x 16 cols = 128 pixels
REG_R, REG_S = 8, 12       # low-res feature region covering one tile
REG_P = REG_R * REG_S      # 96
NT_I, NT_J = 4, 8          # tiles per core: 32 rows/8 x 128 cols/16
N_CORES = 8
BAND = 32                  # output rows per core
NEG = np.float32(-1e4)     # exp(NEG) == 0

CH = 4                     # tiles per chunk (one PSUM bank)
N_CH = NT_I * NT_J // CH   # 8 chunks per core
GC = 2                     # chunks per DMA group
NG = N_CH // GC            # 4 groups (one per ti row-band)

SCALE_PL = (1, 4, 6)       # chunks whose normalize runs on gpsimd

_last_exec_time_ns = None
_cache = {}


def _build_program():
    nc = bacc.Bacc("TRN2", target_bir_lowering=False, debug=False)
    f32 = mybir.dt.float32
    f16 = mybir.dt.float16
    # scattered logits:  [group, region_pix, half*4tiles*128pix]
    wt = nc.dram_tensor("wt", [NG, REG_P, GC * CH * 128], f16,
                        kind="ExternalInput")
    # feature regions:   [group, region_pix, half*4tiles*128chan]
    fg = nc.dram_tensor("fg", [NG, REG_P, GC * CH * 128], f16,
                        kind="ExternalInput")
    # compact logits:    [pixel, chunk*4tiles*25taps]
    mc = nc.dram_tensor("mc", [128, N_CH * CH * MC], f16,
                        kind="ExternalInput")
    # output:            [group, pixel, half*4tiles*128chan]
    out = nc.dram_tensor("out", [NG, 128, GC * CH * 128], f16,
                         kind="ExternalOutput")

    with tile.TileContext(nc) as tc, ExitStack() as ctx:
        wt_pool = ctx.enter_context(tc.tile_pool(name="wt", bufs=NG))
        ew_pool = ctx.enter_context(tc.tile_pool(name="ew", bufs=3))
        fr_pool = ctx.enter_context(tc.tile_pool(name="fr", bufs=NG))
        st_pool = ctx.enter_context(tc.tile_pool(name="st", bufs=2))
        ps_pool = ctx.enter_context(
            tc.tile_pool(name="ps", bufs=2, space=bass.MemorySpace.PSUM))
        cm_pool = ctx.enter_context(tc.tile_pool(name="cm", bufs=1))

        # --- denominator pipeline (compact path) ---
        mcb = cm_pool.tile([128, N_CH, CH, MC], f16)
        nc.sync.dma_start(mcb[:], mc.ap())

        wtb = [None] * NG
        frb = [None] * NG
        for g in range(NG):
            wtb[g] = wt_pool.tile([REG_P, GC, CH * 128], f16,
                                  name="wtb", tag="wtb")
            nc.sync.dma_start(wtb[g][:], wt[g])
            frb[g] = fr_pool.tile([REG_P, GC, CH, 128], f16,
                                  name="frb", tag="frb")
            nc.gpsimd.dma_start(frb[g][:], fg[g])

        mce = cm_pool.tile([128, N_CH, CH, MC], f16)
        nc.scalar.activation(mce[:], mcb[:],
                             mybir.ActivationFunctionType.Exp)
        den = cm_pool.tile([128, N_CH, CH, 1], f32)
        nc.vector.tensor_reduce(den[:], mce[:],
                                axis=mybir.AxisListType.X,
                                op=mybir.AluOpType.add)
        sinv = cm_pool.tile([128, N_CH, CH, 1], f32)
        nc.vector.reciprocal(sinv[:], den[:])

        for g in range(NG):
            ewb = ew_pool.tile([REG_P, GC, CH * 128], f16)
            nc.scalar.activation(ewb[:], wtb[g][:],
                                 mybir.ActivationFunctionType.Exp)
            stage = st_pool.tile([128, GC, CH, 128], f16)
            ps = ps_pool.tile([128, GC, CH, 128], f32)
            for h in range(GC):
                for tjj in range(CH):
                    nc.tensor.matmul(ps[:, h, tjj, :],
                                     ewb[:, h, 128 * tjj:128 * tjj + 128],
                                     frb[g][:, h, tjj, :])
            sb = sinv[:, GC * g:GC * g + GC].broadcast_to(
                (128, GC, CH, 128))
            nc.vector.tensor_mul(stage[:], ps[:], sb)
            if g < 2:
                nc.gpsimd.dma_start(out[g], stage[:])
            else:
                nc.scalar.dma_start(out[g], stage[:])

    nc.compile()
    return nc


def _scatter_indices():
    """Static (p, x) -> mask-channel map for one 8x16 tile.

    p = rr*12+ss indexes the 8x12 feature region, x = u*16+v the output
    pixel. Tap (di,dj) of pixel (u,v) reads region pixel
    (u//2+di, v//2+dj), so channel k = 5*di+dj lands at that p.
    """
    p = np.arange(REG_P)
    rr, ss = p // REG_S, p % REG_S
    x = np.arange(TILE_U * TILE_V)
    u, v = x // TILE_V, x % TILE_V
    di = rr[:, None] - (u[None, :] // 2)
    dj = ss[:, None] - (v[None, :] // 2)
    valid = (di >= 0) & (di < K5) & (dj >= 0) & (dj < K5)
    kidx = np.where(valid, di * K5 + dj, 0)
    return valid, kidx, np.broadcast_to(x, (REG_P, TILE_U * TILE_V))


def _prep_inputs(features, masks):
    features = np.ascontiguousarray(features, dtype=np.float32)
    masks = np.ascontiguousarray(masks, dtype=np.float32)

    # --- weights: scatter mask logits into the per-tile [96, 128] layout
    valid, kidx, xgrid = _scatter_indices()
    # masks -> (b, TI, u, TJ, v, k) -> (b, TI, TJ, x, k)
    mt = masks.reshape(B, H // TILE_U, TILE_U, NT_J, TILE_V, MC)
    mt = mt.transpose(0, 1, 3, 2, 4, 5).reshape(
        B, H // TILE_U, NT_J, TILE_U * TILE_V, MC)
    wt_all = mt[:, :, :, xgrid, kidx]          # [B, 16, TJ, 96, 128]
    wt_all = np.where(valid, wt_all, NEG).astype(np.float32)
    # -> [B, 16, 96, TJ, 128] so each ti band is one contiguous chunk
    wt_all = np.ascontiguousarray(wt_all.transpose(0, 1, 3, 2, 4))

    # --- feature regions (zero-padded borders)
    fpad = np.zeros((B, LH + 4, LW + 4, C), np.float32)
    fpad[:, 2:2 + LH, 2:2 + LW] = features
    p = np.arange(REG_P)
    ti_g = np.arange(H // TILE_U)
    tj_g = np.arange(NT_J)
    ridx = 4 * ti_g[:, None, None] + (p // REG_S)[None, :, None]  # [16,96,1]
    sidx = 8 * tj_g[None, None, :] + (p % REG_S)[None, :, None]   # [1,96,8]
    freg_all = fpad[:, ridx, sidx]             # [B, 16, 96, 8, 128]

    in_maps = []
    for core in range(N_CORES):
        b, band = divmod(core, N_CORES // B)
        # [4ti, 96, 8tj, 128] -> groups g=ti: [4, 96, 2, 512]
        wt_c = np.ascontiguousarray(
            wt_all[b, 4 * band:4 * band + 4].reshape(
                NG, REG_P, GC, CH * 128).astype(np.float16)).reshape(
            NG, REG_P, GC * CH * 128)
        fr_c = np.ascontiguousarray(
            freg_all[b, 4 * band:4 * band + 4].reshape(
                NG, REG_P, GC, CH * 128).astype(np.float16)).reshape(
            NG, REG_P, GC * CH * 128)
        # compact logits: mt[b, ti, tj, x, k] -> [x, ci=ti*2+h, tjj, k]
        mtb = mt[b, 4 * band:4 * band + 4]     # [4ti, 8tj, 128x, 25]
        mcc = np.ascontiguousarray(
            mtb.reshape(NT_I, GC, CH, 128, MC).transpose(3, 0, 1, 2, 4)
            .astype(np.float16)).reshape(128, N_CH * CH * MC)
        in_maps.append({"wt": wt_c, "fg": fr_c, "mc": mcc})
    return in_maps


def kernel(features, masks):
    global _last_exec_time_ns
    if "nc" not in _cache:
        _cache["nc"] = _build_program()
    nc = _cache["nc"]

    in_maps = _prep_inputs(features, masks)
    trace = bool(os.environ.get("CARAFE_TRACE"))
    try:
        res = bass_utils.run_bass_kernel_spmd(
            nc, in_maps, core_ids=list(range(N_CORES)), trace=trace)
    except Exception:
        if not trace:
            raise
        res = bass_utils.run_bass_kernel_spmd(
            nc, in_maps, core_ids=list(range(N_CORES)), trace=False)
    _last_exec_time_ns = res.exec_time_ns
    globals()["_last_result"] = res

    out = np.empty((B, H, W, C), np.float32)
    for core in range(N_CORES):
        b, band = divmod(core, N_CORES // B)
        o = res.results[core]["out"]           # [g, x, h, tjj, c] f16
        o = o.reshape(NG, TILE_U, TILE_V, GC, CH, C)
        o = o.transpose(0, 1, 3, 4, 2, 5).reshape(BAND, W, C)
        out[b, BAND * band:BAND * band + BAND] = o.astype(np.float32)
    return out


# revision 10
# speedup vs baseline: 1.1470x; 1.1470x over previous
"""CARAFE (content-aware upsampling) Trainium2 Bass kernel.

Problem: features [2,64,64,128] f32, masks [2,128,128,25] f32 ->
out [2,128,128,128] f32; kernel_size=5, 2x nearest upsample, per-pixel
softmax over the 25-tap window, weighted sum of the 5x5 low-res patch.

Formulation: for each 8x16 output-pixel tile the 25 taps of all 128
pixels live inside an 8x12 low-res feature region (96 pixels). The
whole tile is then ONE matmul on the tensor engine:

    out[pix, c] = sum_p expW[p, pix] * Freg[p, c] / denom[pix]

where expW is the exp of the raw mask logits scattered (host-side, pure
data movement) into the [96 region, 128 pix] layout with -1e4 fill
(exp -> 0). exp runs on the scalar engine over the scattered layout.
Denominators come from a separately-loaded COMPACT copy of the logits
([pixel, 25]) which is exp'd (one scalar instr), free-dim-reduced and
reciprocal'd on the vector engine -- identical f16 exp values, so the
softmax is exact.  The PSUM->SBUF normalize is one broadcast
tensor_tensor multiply per 4-tile chunk (f32 PSUM read, f16 write),
split between the vector and gpsimd engines to balance load.

All DRAM traffic is host-prearranged fully contiguous and f16 (output
converted to f32 host-side): per core 1 compact-mask load (205KB),
4 weight loads (197KB), 4 feature-region loads (197KB), 4 output
stores (256KB).

Sharding: 8 cores = batch (2) x 4 row-bands of 32 output rows.
"""

import os
import numpy as np
from contextlib import ExitStack

import concourse.bacc as bacc
import concourse.bass as bass
import concourse.tile as tile
import concourse.mybir as mybir
from concourse import bass_utils

B, H, W, MC = 2, 128, 128, 25
LH, LW, C = 64, 64, 128
K5 = 5
TILE_U, TILE_V = 8, 16     # output tile: 8 rows x 16 cols = 128 pixels
REG_R, REG_S = 8, 12       # low-res feature region covering one tile
REG_P = REG_R * REG_S      # 96
NT_I, NT_J = 4, 8          # tiles per core: 32 rows/8 x 128 cols/16
N_CORES = 8
BAND = 32                  # output rows per core
NEG = np.float32(-1e4)     # exp(NEG) == 0

CH = 4                     # tiles per chunk (one PSUM bank)
N_CH = NT_I * NT_J // CH   # 8 chunks per core
GC = 2                     # chunks per DMA group
NG = N_CH // GC            # 4 groups (one per ti row-band)

SCALE_PL = (1, 4, 6)       # chunks whose normalize runs on gpsimd

_last_exec_time_ns = None
_cache = {}


def _build_program():
    nc = bacc.Bacc("TRN2", target_bir_lowering=False, debug=False)
    f32 = mybir.dt.float32
    f16 = mybir.dt.float16
    # scattered logits + feature regions, one fused tensor per group:
    # [group, region_pix, 0=logits/1=features, half*4tiles*128]
    wf = nc.dram_tensor("wf", [NG, REG_P, 2, GC * CH * 128], f16,
                        kind="ExternalInput")
    # compact logits:    [pixel, chunk*4tiles*25taps]
    mc = nc.dram_tensor("mc", [128, N_CH * CH * MC], f16,
                        kind="ExternalInput")
    # output:            [group, pixel, half*4tiles*128chan]
    out = nc.dram_tensor("out", [NG, 128, GC * CH * 128], f16,
                         kind="ExternalOutput")

    with tile.TileContext(nc) as tc, ExitStack() as ctx:
        wf_pool = ctx.enter_context(tc.tile_pool(name="wf", bufs=NG))
        ew_pool = ctx.enter_context(tc.tile_pool(name="ew", bufs=3))
        st_pool = ctx.enter_context(tc.tile_pool(name="st", bufs=NG))
        ps_pool = ctx.enter_context(
            tc.tile_pool(name="ps", bufs=3, space=bass.MemorySpace.PSUM))
        cm_pool = ctx.enter_context(tc.tile_pool(name="cm", bufs=1))

        # --- denominator pipeline (compact path); mc rides the scalar
        # HWDGE queue so it lands before the first wf group on sync's.
        mcb = cm_pool.tile([128, N_CH, CH, MC], f16)
        nc.scalar.dma_start(mcb[:], mc.ap())

        wfb = [None] * NG
        for g in range(NG):
            wfb[g] = wf_pool.tile([REG_P, 2, GC, CH * 128], f16,
                                  name="wfb", tag="wfb")
            nc.sync.dma_start(wfb[g][:], wf[g])

        mce = cm_pool.tile([128, N_CH, CH, MC], f16)
        nc.scalar.activation(mce[:], mcb[:],
                             mybir.ActivationFunctionType.Exp)
        den = cm_pool.tile([128, N_CH, CH, 1], f32)
        nc.vector.tensor_reduce(den[:], mce[:],
                                axis=mybir.AxisListType.X,
                                op=mybir.AluOpType.add)
        sinv = cm_pool.tile([128, N_CH, CH, 1], f32)
        nc.vector.reciprocal(sinv[:], den[:])

        for g in range(NG):
            ewb = ew_pool.tile([REG_P, GC, CH * 128], f16)
            nc.scalar.activation(ewb[:], wfb[g][:, 0],
                                 mybir.ActivationFunctionType.Exp)
            stage = st_pool.tile([128, GC, CH, 128], f16)
            ps = ps_pool.tile([128, GC, CH, 128], f32)
            for h in range(GC):
                for tjj in range(CH):
                    nc.tensor.matmul(ps[:, h, tjj, :],
                                     ewb[:, h, 128 * tjj:128 * tjj + 128],
                                     wfb[g][:, 1, h,
                                            128 * tjj:128 * tjj + 128])
            sb = sinv[:, GC * g:GC * g + GC].broadcast_to(
                (128, GC, CH, 128))
            nc.vector.tensor_mul(stage[:], ps[:], sb)
            if g < NG - 1:
                nc.gpsimd.dma_start(out[g], stage[:])
            else:
                nc.scalar.dma_start(out[g], stage[:])

    nc.compile()
    return nc


def _scatter_indices():
    """Static (p, x) -> mask-channel map for one 8x16 tile.

    p = rr*12+ss indexes the 8x12 feature region, x = u*16+v the output
    pixel. Tap (di,dj) of pixel (u,v) reads region pixel
    (u//2+di, v//2+dj), so channel k = 5*di+dj lands at that p.
    """
    p = np.arange(REG_P)
    rr, ss = p // REG_S, p % REG_S
    x = np.arange(TILE_U * TILE_V)
    u, v = x // TILE_V, x % TILE_V
    di = rr[:, None] - (u[None, :] // 2)
    dj = ss[:, None] - (v[None, :] // 2)
    valid = (di >= 0) & (di < K5) & (dj >= 0) & (dj < K5)
    kidx = np.where(valid, di * K5 + dj, 0)
    return valid, kidx, np.broadcast_to(x, (REG_P, TILE_U * TILE_V))


def _prep_inputs(features, masks):
    features = np.ascontiguousarray(features, dtype=np.float32)
    masks = np.ascontiguousarray(masks, dtype=np.float32)

    # --- weights: scatter mask logits into the per-tile [96, 128] layout
    valid, kidx, xgrid = _scatter_indices()
    # masks -> (b, TI, u, TJ, v, k) -> (b, TI, TJ, x, k)
    mt = masks.reshape(B, H // TILE_U, TILE_U, NT_J, TILE_V, MC)
    mt = mt.transpose(0, 1, 3, 2, 4, 5).reshape(
        B, H // TILE_U, NT_J, TILE_U * TILE_V, MC)
    wt_all = mt[:, :, :, xgrid, kidx]          # [B, 16, TJ, 96, 128]
    wt_all = np.where(valid, wt_all, NEG).astype(np.float32)
    # -> [B, 16, 96, TJ, 128] so each ti band is one contiguous chunk
    wt_all = np.ascontiguousarray(wt_all.transpose(0, 1, 3, 2, 4))

    # --- feature regions (zero-padded borders)
    fpad = np.zeros((B, LH + 4, LW + 4, C), np.float32)
    fpad[:, 2:2 + LH, 2:2 + LW] = features
    p = np.arange(REG_P)
    ti_g = np.arange(H // TILE_U)
    tj_g = np.arange(NT_J)
    ridx = 4 * ti_g[:, None, None] + (p // REG_S)[None, :, None]  # [16,96,1]
    sidx = 8 * tj_g[None, None, :] + (p % REG_S)[None, :, None]   # [1,96,8]
    freg_all = fpad[:, ridx, sidx]             # [B, 16, 96, 8, 128]

    in_maps = []
    for core in range(N_CORES):
        b, band = divmod(core, N_CORES // B)
        # [4ti, 96, 8tj, 128] -> groups g=ti: [4, 96, 512*GC]
        wt_c = wt_all[b, 4 * band:4 * band + 4].reshape(
            NG, REG_P, 1, GC * CH * 128)
        fr_c = freg_all[b, 4 * band:4 * band + 4].reshape(
            NG, REG_P, 1, GC * CH * 128)
        wf_c = np.ascontiguousarray(
            np.concatenate([wt_c, fr_c], axis=2).astype(np.float16))
        # compact logits: mt[b, ti, tj, x, k] -> [x, ci=ti*2+h, tjj, k]
        mtb = mt[b, 4 * band:4 * band + 4]     # [4ti, 8tj, 128x, 25]
        mcc = np.ascontiguousarray(
            mtb.reshape(NT_I, GC, CH, 128, MC).transpose(3, 0, 1, 2, 4)
            .astype(np.float16)).reshape(128, N_CH * CH * MC)
        in_maps.append({"wf": wf_c, "mc": mcc})
    return in_maps


def kernel(features, masks):
    global _last_exec_time_ns
    if "nc" not in _cache:
        _cache["nc"] = _build_program()
    nc = _cache["nc"]

    in_maps = _prep_inputs(features, masks)
    trace = bool(os.environ.get("CARAFE_TRACE"))
    try:
        res = bass_utils.run_bass_kernel_spmd(
            nc, in_maps, core_ids=list(range(N_CORES)), trace=trace)
    except Exception:
        if not trace:
            raise
        res = bass_utils.run_bass_kernel_spmd(
            nc, in_maps, core_ids=list(range(N_CORES)), trace=False)
    _last_exec_time_ns = res.exec_time_ns
    globals()["_last_result"] = res

    out = np.empty((B, H, W, C), np.float32)
    for core in range(N_CORES):
        b, band = divmod(core, N_CORES // B)
        o = res.results[core]["out"]           # [g, x, h, tjj, c] f16
        o = o.reshape(NG, TILE_U, TILE_V, GC, CH, C)
        o = o.transpose(0, 1, 3, 4, 2, 5).reshape(BAND, W, C)
        out[b, BAND * band:BAND * band + BAND] = o.astype(np.float32)
    return out
